# revision 86
# baseline (speedup 1.0000x reference)
"""Two-layer GAT (DGL GATConv) on 8 Trainium2 NeuronCores.

Edge-parallel by dst node (core c owns dst nodes [6250c, 6250(c+1))), with
ONE shared gather schedule for both layers:
  * Table row order follows the AllGather chunk layout (AG_CHUNKS): within
    each chunk, per-core shards are concatenated, so collective outputs are
    directly gather-addressable.  The A section (own tiles 0-29) and B
    section (30-49) are each <32768 rows so int16 gather indices reach
    either via an A/B split; every 128-row dst window is one table tile.
  * Layer 1: x is replicated (host-transposed bf16 xT_all in table-row
    order) and every core projects the FULL table locally — no collective.
    Rows are 256B [fs fp8(128B, d-major) | el bf16(8B) | junk].
  * Layer 2: own nodes projected inline after each layer-1 group epilogue;
    rows 256B [fs2 fp8(188B, c-major) | el2 bf16(8B) | junk].  The table is
    AllGathered in four chunks (10/10/10/20 tiles) issued mid-stream so the
    collectives overlap layer-1 edge compute; layer-2 runs as two passes
    (A-half chunks, then B-half) accumulating through a bf16 segbuf so the
    A pass overlaps the last collective.
  * Edge phase per group of 128 dst nodes: dma_gathers by src row (256B
    elems), then one-hot scatter matmuls into PSUM:
      - ohT[m,(k,j)] = (drep==m), batched 4x DVE tensor_scalar,
      - er per edge via per-chunk matmul ohT.T @ erw,
      - ee = exp(lrelu(el+er)) on ACT; mt = [ee*fs | ee] (d-major 2x mult),
      - seg += oh_c.T @ mt_c, oh built batched as oh_mc[j,m,c] (one 2x
        TensorTensor per group, strided lhsT slices per chunk).
  * Feature order is d-major (head fastest) everywhere so every per-head
    broadcast multiply has a packed 2-byte inner axis (DVE 2x/4x modes);
    fp8 table reads are widened to bf16 on the Activation engine.

Softmax max-subtraction is skipped: |logits| < ~10 with these scales and
alpha is shift-invariant.  Host work is schedule/index prep and constant
reparameterization (W @ blockdiag(a), permutations, x transpose) only.
"""

import math
import os
import sys
from contextlib import ExitStack

import numpy as np

for _p in ("/opt/trn_rl_repo", "/root/.axon_site/_ro/trn_rl_repo"):
    if os.path.isdir(_p) and _p not in sys.path:
        sys.path.append(_p)

import ml_dtypes

import concourse.bass as bass
import concourse.tile as tile
from concourse import bacc, mybir
from concourse.bass_utils import run_bass_kernel_spmd

BF16 = ml_dtypes.bfloat16
FP8 = ml_dtypes.float8_e4m3

N = 50000
E = 800000
F_IN = 128
H, D, C = 4, 32, 47
HD = H * D            # 128
HC = H * C            # 188
NEG_SLOPE = 0.2

NCORES = 8
P = 128
NPC = N // NCORES         # 6250
HALF_A = 3840             # per-core shard low half (30 tiles of 128)
HALF_B = 2560             # per-core shard high half (20 tiles)
NPAD = HALF_A + HALF_B    # 6400 padded rows per core
S_A = NCORES * HALF_A     # 30720 rows in table section A
S_B = NCORES * HALF_B     # 20480 rows in table section B
NTAB = S_A + S_B          # 51200
TILES = NTAB // P         # 400 projection tiles
G = math.ceil(NPC / P)    # 49 dst groups per core
NOUT = G * P              # 6272 output rows per core

W1 = HD + H               # 132 cols used in layer-1 rows (fs|el)
W2 = HC + H               # 192 (fs2|el2)
ELEM1 = 256               # layer-1 table row bytes (uint8) -> 256B
ELEM2 = 256               # layer-2 table row bytes (uint8) -> 256B
OOR = 200.0               # out-of-window dst_local marker

LAST_EXEC_NS = None
_LAST_SCHED = None


# AllGather chunk boundaries in own-tile units: A-section is gathered in
# three chunks, so each chunk concatenates per-core shards independently.
AG_CHUNKS = [(0, 10), (10, 20), (20, 30), (30, 50)]


def _row_of(v):
    c = v // NPC
    r = v % NPC
    conds, vals = [], []
    base = 0
    for (t0, t1) in AG_CHUNKS:
        lo, hi, rpc = t0 * P, t1 * P, (t1 - t0) * P
        conds.append((r >= lo) & (r < hi))
        vals.append(base + c * rpc + (r - lo))
        base += NCORES * rpc
    return np.select(conds, vals)


def _schedule(src, dst):
    """Edge schedule shared by both layers. Returns dict of arrays + KA/KB."""
    order = np.argsort(dst, kind="stable")
    s_src = src[order].astype(np.int64)
    s_dst = dst[order].astype(np.int64)

    core_of = s_dst // NPC
    rl = s_dst % NPC
    g_of = rl // P
    win = rl % P
    rowsrc = _row_of(s_src)
    half = (rowsrc >= S_A).astype(np.int64)

    cgh = (core_of * G + g_of) * 2 + half
    order2 = np.argsort(cgh, kind="stable")
    cgh = cgh[order2]
    rowsrc = rowsrc[order2]
    win = win[order2]

    counts = np.bincount(cgh, minlength=NCORES * G * 2).reshape(NCORES, G, 2)
    KA = int(math.ceil(counts[:, :, 0].max() / P))
    KB = int(math.ceil(counts[:, :, 1].max() / P))
    K = KA + KB

    starts = np.zeros(NCORES * G * 2 + 1, dtype=np.int64)
    np.cumsum(counts.ravel(), out=starts[1:])
    pos_in_run = np.arange(len(cgh)) - starts[cgh]

    base = np.where(cgh % 2 == 0, 0, KA * P)
    flat = base + pos_in_run
    cg = cgh // 2

    idx_flat = np.zeros((NCORES * G, K * P), dtype=np.int64)   # dummy row 0
    dstl_flat = np.full((NCORES * G, K * P), OOR, dtype=np.float32)
    idx_val = np.where(cgh % 2 == 0, rowsrc, rowsrc - S_A)
    idx_flat[cg, flat] = idx_val
    dstl_flat[cg, flat] = win

    idx_flat = idx_flat.reshape(NCORES, G, K, P)
    dstl_flat = dstl_flat.reshape(NCORES, G, K, P)

    def wrap(a):
        # [.., n] flat slot-major -> [.., 128, n/16] wrapped+replicated
        n = a.shape[-1]
        w = a.reshape(*a.shape[:-1], n // 16, 16)
        w = np.swapaxes(w, -1, -2)                    # [16, n/16]
        return np.tile(w, (1, 1, 8, 1)).astype(np.int16)  # [128, n/16]

    idxA_w = wrap(idx_flat[:, :, :KA, :].reshape(NCORES, G, KA * P))
    idxB_w = wrap(idx_flat[:, :, KA:, :].reshape(NCORES, G, KB * P))

    dstl_row = dstl_flat.reshape(NCORES, G, K * P).astype(BF16)
    dstl_col = np.ascontiguousarray(
        np.swapaxes(dstl_flat, 2, 3)).astype(BF16)     # [NC, G, 128, K]
    # one combined per-group [P, KA*8 + KB*8 + K] int16 schedule load
    sched = np.concatenate(
        [idxA_w, idxB_w, dstl_col.view(np.int16)], axis=3)
    return dict(sched=sched, dstl_row=dstl_row, KA=KA, KB=KB)


def _blockdiag(a, hd, h, dim):
    out = np.zeros((hd, h), dtype=np.float32)
    for i in range(h):
        out[i * dim:(i + 1) * dim, i] = a[i]
    return out


def _build_program(KA, KB):
    K = KA + KB
    SCHED_W = KA * 8 + KB * 8 + K
    nc = bacc.Bacc("TRN2", target_bir_lowering=False, debug=False,
                   num_devices=NCORES, num_swdge_queues=2)
    dt = mybir.dt
    f32, bf16, i16, u8 = dt.float32, dt.bfloat16, dt.int16, dt.uint8
    fp8 = dt.float8e4

    def inp(name, shape, d=f32):
        return nc.dram_tensor(name, shape, d, kind="ExternalInput").ap()

    xT_all = inp("xT_all", [P, NTAB], bf16)      # replicated, row order
    xT_own = inp("xT_own", [P, NPAD], bf16)      # own nodes, local order
    w1cat = inp("w1cat", [F_IN, W1 + H], bf16)   # fs(d-major)|el|er
    w2cat = inp("w2cat", [F_IN, W2 + H], bf16)   # fs2(c-major)|el2|er2
    b1_t = inp("b1_t", [P, HD])
    b2m_t = inp("b2m_t", [P, C])
    iota_r = inp("iota_r", [P, P], bf16)         # iota_r[p, m] = m
    iota_mc_in = inp("iota_mc", [P, P * (KA + KB)], bf16)  # [p, m*K+c] = m
    iota_c = inp("iota_c", [P, 1])               # iota_c[p, 0] = p (f32)
    ident_t = inp("ident_t", [P, P], bf16)
    sched_in = inp("sched_in", [G, P, SCHED_W], i16)
    dstl_row_in = inp("dstl_row_in", [G, K * P], bf16)

    y_out = nc.dram_tensor("y_out", [NOUT, C], f32, kind="ExternalOutput").ap()

    tab1A = nc.dram_tensor("tab1A", [S_A, ELEM1], u8).ap()
    tab1B = nc.dram_tensor("tab1B", [S_B, ELEM1], u8).ap()
    tab2_ownA1 = nc.dram_tensor("tab2_ownA1", [10 * P, ELEM2], u8).ap()
    tab2_ownA2 = nc.dram_tensor("tab2_ownA2", [10 * P, ELEM2], u8).ap()
    tab2_ownA3 = nc.dram_tensor("tab2_ownA3", [10 * P, ELEM2], u8).ap()
    tab2_ownB = nc.dram_tensor("tab2_ownB", [HALF_B, ELEM2], u8).ap()
    tab2A = nc.dram_tensor("tab2A", [S_A, ELEM2], u8,
                           addr_space="Shared").ap()
    tab2B = nc.dram_tensor("tab2B", [S_B, ELEM2], u8,
                           addr_space="Shared").ap()

    with tile.TileContext(nc) as tc, ExitStack() as ctx:
        const = ctx.enter_context(tc.tile_pool(name="const", bufs=1))
        sb = ctx.enter_context(tc.tile_pool(name="sb", bufs=3))
        gat = ctx.enter_context(tc.tile_pool(name="gat", bufs=3))
        ps = ctx.enter_context(tc.tile_pool(name="ps", bufs=2, space="PSUM"))
        psg = ctx.enter_context(tc.tile_pool(name="psg", bufs=3, space="PSUM"))
        big = ctx.enter_context(tc.tile_pool(name="big", bufs=1))

        iota = const.tile([P, P], bf16)
        nc.sync.dma_start(iota[:], iota_r[:])
        iota_mc = const.tile([P, P * K], bf16)
        nc.sync.dma_start(iota_mc[:], iota_mc_in[:])
        iotac = const.tile([P, 1], f32)
        nc.sync.dma_start(iotac[:], iota_c[:])
        ident = const.tile([P, P], bf16)
        nc.sync.dma_start(ident[:], ident_t[:])
        b1s = const.tile([P, HD], f32)
        nc.sync.dma_start(b1s[:], b1_t[:])
        b2ms = const.tile([P, C], f32)
        nc.sync.dma_start(b2ms[:], b2m_t[:])
        w1 = const.tile([P, W1 + H], bf16)
        nc.sync.dma_start(w1[:], w1cat[:])
        w2 = const.tile([P, W2 + H], bf16)
        nc.sync.dma_start(w2[:], w2cat[:])


        er1 = big.tile([P, G, H], bf16)
        er2 = big.tile([P, G, H], bf16)
        zs = big.tile([P, G, C], f32)
        ss = big.tile([P, G], f32)

        # ---------------- layer-1 projection: full table ----------------
        TB = 12  # tiles per DMA batch (4 PSUM sub-batches of 3)
        PB = 3
        TA = S_A // P
        for t0 in [*range(0, TA, TB), *range(TA, TILES, TB)]:
            tb = min(TB, (TA if t0 < TA else TILES) - t0)
            xt = sb.tile([P, TB, P], bf16, tag="xload")
            nc.sync.dma_start(xt[:, :tb, :],
                              xT_all[:, t0 * P:(t0 + tb) * P].rearrange(
                                  "p (t q) -> p t q", t=tb))
            rows = sb.tile([P, TB, HD + 2 * H], u8, tag="rows")
            for j0 in range(0, tb, PB):
                jb = min(PB, tb - j0)
                pr3 = ps.tile([P, PB, W1 + H], f32, space="PSUM", tag="mm")
                for i in range(jb):
                    nc.tensor.matmul(pr3[:, i, :], lhsT=xt[:, j0 + i, :],
                                     rhs=w1[:], start=True, stop=True)
                if (t0 + j0) % 2 == 0:
                    nc.scalar.activation(
                        rows[:, j0:j0 + jb, :HD].bitcast(fp8),
                        pr3[:, :jb, :HD],
                        mybir.ActivationFunctionType.Copy)
                    nc.vector.tensor_copy(
                        rows[:, j0:j0 + jb, HD:HD + 2 * H].bitcast(bf16),
                        pr3[:, :jb, HD:HD + H])
                else:
                    nc.vector.tensor_copy(
                        rows[:, j0:j0 + jb, :HD].bitcast(fp8),
                        pr3[:, :jb, :HD])
                    nc.vector.tensor_copy(
                        rows[:, j0:j0 + jb, HD:HD + 2 * H].bitcast(bf16),
                        pr3[:, :jb, HD:HD + H])
            if t0 < S_A // P:
                dst_ap = tab1A[t0 * P:(t0 + tb) * P, :HD + 2 * H]
            else:
                t1 = t0 - S_A // P
                dst_ap = tab1B[t1 * P:(t1 + tb) * P, :HD + 2 * H]
            nc.sync.dma_start(
                dst_ap.rearrange("(t p) w -> p t w", p=P), rows[:, :tb, :])

        # er for own dst windows (local order) from xT_own
        XC = 10
        for g0 in range(0, G, XC):
            gb = min(XC, G - g0)
            xTo = sb.tile([P, XC * P], bf16, tag="xto", bufs=2)
            nc.sync.dma_start(xTo[:, :gb * P],
                              xT_own[:, g0 * P:(g0 + gb) * P])
            pre = ps.tile([P, XC, H], f32, space="PSUM", tag="mm")
            for i in range(gb):
                nc.tensor.matmul(pre[:, i, :], lhsT=xTo[:, i * P:(i + 1) * P],
                                 rhs=w1[:, W1:W1 + H], start=True, stop=True)
            nc.vector.tensor_copy(er1[:, g0:g0 + gb, :], pre[:, :gb, :])

        # ---------------- edge phase ----------------
        def edge_chunks(layer, suf, g, k0, kn, out_cb):
            """Process chunks [k0, k0+kn) of group g into a PSUM seg tile."""
            st = sb.tile([P, SCHED_W], i16, tag=f"sched{suf}", bufs=4)
            nc.sync.dma_start(st[:], sched_in[g])
            dcol = st[:, KA * 8 + KB * 8:].bitcast(bf16)
            drep = sb.tile([P, kn * P], bf16, tag=f"drep{suf}", bufs=3)
            nc.scalar.dma_start(
                drep[:], dstl_row_in[g:g + 1, k0 * P:(k0 + kn) * P]
                .to_broadcast([P, kn * P]))

            if layer == 1:
                tabA, tabB = tab1A, tab1B
                erw = er1[:, g, :]
                width, msg = W1, HD
            else:
                tabA, tabB = tab2A, tab2B
                erw = er2[:, g, :]
                width, msg = W2, HC
            gelem = ELEM1 if layer == 1 else ELEM2
            gt = gat.tile([P, kn, gelem], u8, tag=f"gt{suf}")
            if kn == K:
                nc.gpsimd.dma_gather(
                    out_ap=gt[:, :KA, :], in_ap=tabA[:],
                    idxs_ap=st[:, :KA * 8], num_idxs=KA * P,
                    num_idxs_reg=KA * P, elem_size=gelem,
                    single_packet=False, queue_num=1)
                nc.gpsimd.dma_gather(
                    out_ap=gt[:, KA:, :], in_ap=tabB[:],
                    idxs_ap=st[:, KA * 8:KA * 8 + KB * 8],
                    num_idxs=KB * P,
                    num_idxs_reg=KB * P, elem_size=gelem,
                    single_packet=False, queue_num=1)
            elif k0 == 0:
                nc.gpsimd.dma_gather(
                    out_ap=gt[:], in_ap=tabA[:],
                    idxs_ap=st[:, :KA * 8], num_idxs=KA * P,
                    num_idxs_reg=KA * P, elem_size=gelem,
                    single_packet=False, queue_num=1)
            else:
                nc.gpsimd.dma_gather(
                    out_ap=gt[:], in_ap=tabB[:],
                    idxs_ap=st[:, KA * 8:KA * 8 + KB * 8],
                    num_idxs=KB * P,
                    num_idxs_reg=KB * P, elem_size=gelem,
                    single_packet=False, queue_num=1)
            fs8 = gt[:, :, :msg].bitcast(fp8)
            fsb = sb.tile([P, kn, msg], bf16, tag=f"fsb{suf}", bufs=3)
            nc.scalar.activation(fsb[:], fs8,
                                 mybir.ActivationFunctionType.Copy)
            fs_ap = fsb[:]
            el_ap = gt[:, :, msg:msg + 2 * H].bitcast(bf16)

            # ohT[m, k, j] = (dstl[k*128+j] == m)   (4x tensor_scalar)
            ohT = sb.tile([P, kn, P], bf16, tag=f"ohT{suf}", bufs=3)
            nc.vector.tensor_scalar(
                out=ohT[:],
                in0=drep[:].rearrange("p (k j) -> p k j", k=kn),
                scalar1=iotac[:, :1], scalar2=None,
                op0=mybir.AluOpType.is_equal)

            # er per edge: erp[j, k, h] = sum_m ohT[m,k,j] * erw[m,h]
            erp = psg.tile([P, kn, H], f32, space="PSUM", tag="erp")
            for c in range(kn):
                nc.tensor.matmul(erp[:, c, :], lhsT=ohT[:, c, :],
                                 rhs=erw, start=True, stop=True)

            # ev = lrelu(el + er); mt = [ee*fs | ee]
            ev = sb.tile([P, kn, H], f32, tag=f"ev{suf}")
            nc.vector.tensor_tensor(out=ev[:], in0=el_ap, in1=erp[:],
                                    op=mybir.AluOpType.add)
            nc.vector.scalar_tensor_tensor(
                out=ev[:], in0=ev[:], scalar=NEG_SLOPE, in1=ev[:],
                op0=mybir.AluOpType.mult, op1=mybir.AluOpType.max)
            mt = sb.tile([P, kn, width], bf16, tag=f"mt{suf}", bufs=3)
            nc.scalar.activation(mt[:, :, msg:width], ev[:],
                                 mybir.ActivationFunctionType.Exp)
            nc.vector.tensor_tensor(
                out=mt[:, :, :msg].rearrange(
                    "p k (d h) -> p k d h", h=H),
                in0=fs_ap.rearrange("p k (d h) -> p k d h", h=H),
                in1=mt[:, :, msg:width, None].rearrange(
                    "p k w o -> p k o w").to_broadcast(
                        [P, kn, msg // H, H]),
                op=mybir.AluOpType.mult)

            # oh_mc[j, m, c] = (dstl[c] == m); lhsT slice per chunk
            oh_mc = sb.tile([P, P, kn], bf16, tag=f"oh{suf}", bufs=2)
            nc.vector.tensor_tensor(
                out=oh_mc[:],
                in0=dcol[:, None, k0:k0 + kn].to_broadcast([P, P, kn]),
                in1=iota_mc[:].rearrange("p (m c) -> p m c", c=K)[:, :, :kn],
                op=mybir.AluOpType.is_equal)
            # seg[m, w] += sum_j oh_c[j, m] * mt[j, c, w]
            seg = psg.tile([P, width], f32, space="PSUM", tag="seg")
            for c in range(kn):
                nc.tensor.matmul(seg[:], lhsT=oh_mc[:, :, c],
                                 rhs=mt[:, c, :],
                                 start=(c == 0), stop=(c == kn - 1))
            out_cb(g, seg)

        # ---------------- layer-1 epilogue + inline proj2 ----------------
        segbuf = big.tile([P, G, W2], bf16)

        def l1_out(g, seg):
            dn = sb.tile([P, H], f32, tag="dn")
            nc.vector.tensor_scalar_max(dn[:], seg[:, HD:HD + H], 1e-30)
            rd = sb.tile([P, H], f32, tag="rd")
            nc.vector.reciprocal(rd[:], dn[:])
            ht = sb.tile([P, F_IN], f32, tag="ht")
            nc.vector.tensor_tensor(
                out=ht[:].rearrange("p (d h) -> p d h", h=H),
                in0=seg[:, :HD].rearrange("p (d h) -> p d h", h=H),
                in1=rd[:, None, :].to_broadcast([P, D, H]),
                op=mybir.AluOpType.mult)
            nc.vector.tensor_tensor(
                out=ht[:], in0=ht[:], in1=b1s[:], op=mybir.AluOpType.add)
            mn = sb.tile([P, F_IN], f32, tag="mn")
            nc.vector.tensor_scalar_min(mn[:], ht[:], 0.0)
            nc.scalar.activation(mn[:], mn[:], mybir.ActivationFunctionType.Exp)
            h1g = sb.tile([P, F_IN], bf16, tag="h1g")
            nc.vector.scalar_tensor_tensor(
                out=h1g[:], in0=mn[:], scalar=-1.0, in1=ht[:],
                op0=mybir.AluOpType.add, op1=mybir.AluOpType.max)

            # inline layer-2 projection of this dst window
            hT_ps = ps.tile([F_IN, P], bf16, space="PSUM", tag="mm")
            nc.tensor.transpose(hT_ps[:], h1g[:], ident[:])
            hT = sb.tile([F_IN, P], bf16, tag="hT")
            nc.vector.tensor_copy(hT[:], hT_ps[:])
            pr2 = ps.tile([P, W2 + H], f32, space="PSUM", tag="mm")
            nc.tensor.matmul(pr2[:], lhsT=hT[:], rhs=w2[:],
                             start=True, stop=True)
            nc.vector.tensor_copy(er2[:, g, :], pr2[:, W2:W2 + H])
            row2 = sb.tile([P, ELEM2], u8, tag="row2")
            nc.vector.tensor_copy(row2[:, :HC].bitcast(fp8), pr2[:, :HC])
            nc.vector.tensor_copy(row2[:, HC:HC + 2 * H].bitcast(bf16),
                                  pr2[:, HC:HC + H])
            if g < 10:
                nc.sync.dma_start(tab2_ownA1[g * P:(g + 1) * P, :], row2[:])
            elif g < 20:
                g2 = g - 10
                nc.sync.dma_start(tab2_ownA2[g2 * P:(g2 + 1) * P, :], row2[:])
            elif g < 30:
                g2 = g - 20
                nc.sync.dma_start(tab2_ownA3[g2 * P:(g2 + 1) * P, :], row2[:])
            else:
                g2 = g - 30
                nc.sync.dma_start(tab2_ownB[g2 * P:(g2 + 1) * P, :], row2[:])

        for g in range(10):
            edge_chunks(1, "1", g, 0, K, l1_out)

        nc.gpsimd.collective_compute(
            "AllGather", mybir.AluOpType.bypass,
            replica_groups=[list(range(NCORES))],
            ins=[tab2_ownA1[:]], outs=[tab2A[:NCORES * 10 * P, :]])

        for g in range(10, 20):
            edge_chunks(1, "1", g, 0, K, l1_out)

        nc.gpsimd.collective_compute(
            "AllGather", mybir.AluOpType.bypass,
            replica_groups=[list(range(NCORES))],
            ins=[tab2_ownA2[:]],
            outs=[tab2A[NCORES * 10 * P:NCORES * 20 * P, :]])

        for g in range(20, 30):
            edge_chunks(1, "1", g, 0, K, l1_out)

        nc.gpsimd.collective_compute(
            "AllGather", mybir.AluOpType.bypass,
            replica_groups=[list(range(NCORES))],
            ins=[tab2_ownA3[:]], outs=[tab2A[NCORES * 20 * P:, :]])

        for g in range(30, G):
            edge_chunks(1, "1", g, 0, K, l1_out)

        nc.gpsimd.collective_compute(
            "AllGather", mybir.AluOpType.bypass,
            replica_groups=[list(range(NCORES))],
            ins=[tab2_ownB[:]], outs=[tab2B[:]])

        # ---------------- layer-2 epilogue ----------------
        def l2a_out(g, seg):
            nc.scalar.activation(segbuf[:, g, :], seg[:],
                                 mybir.ActivationFunctionType.Copy)

        def l2_out(g, seg_ps):
            seg = sb.tile([P, W2], f32, tag="segf")
            nc.vector.tensor_tensor(out=seg[:], in0=seg_ps[:],
                                    in1=segbuf[:, g, :],
                                    op=mybir.AluOpType.add)
            dn = sb.tile([P, H], f32, tag="dn2")
            nc.vector.tensor_scalar_max(dn[:], seg[:, HC:HC + H], 1e-30)
            rd = sb.tile([P, H], f32, tag="rd2")
            nc.vector.reciprocal(rd[:], dn[:])
            nc.vector.tensor_scalar_mul(rd[:], rd[:], 1.0 / H)
            z = sb.tile([P, HC], f32, tag="z")
            nc.vector.tensor_tensor(
                out=z[:].rearrange("p (c h) -> p c h", h=H),
                in0=seg[:, :HC].rearrange("p (c h) -> p c h", h=H),
                in1=rd[:, None, :].to_broadcast([P, C, H]),
                op=mybir.AluOpType.mult)
            z4 = sb.tile([P, C], f32, tag="z4")
            nc.vector.reduce_sum(
                z4[:], z[:].rearrange("p (c h) -> p c h", h=H),
                axis=mybir.AxisListType.X)
            nc.vector.tensor_tensor(
                out=z4[:], in0=z4[:], in1=b2ms[:], op=mybir.AluOpType.add)
            zm = sb.tile([P, 1], f32, tag="zm")
            nc.vector.reduce_max(zm[:], z4[:], axis=mybir.AxisListType.X)
            nc.vector.tensor_scalar(
                out=zs[:, g, :], in0=z4[:], scalar1=zm[:, :1], scalar2=None,
                op0=mybir.AluOpType.subtract)
            es = sb.tile([P, C], f32, tag="es")
            nc.scalar.activation(es[:], zs[:, g, :],
                                 mybir.ActivationFunctionType.Exp,
                                 accum_out=ss[:, g:g + 1])

        tc.no_sync_barrier()
        for g in range(G):
            edge_chunks(2, "2", g, 0, KA, l2a_out)

        tc.no_sync_barrier()
        for g in range(G):
            edge_chunks(2, "2", g, KA, KB, l2_out)

        lg = sb.tile([P, G], f32, tag="lg")
        nc.scalar.activation(lg[:], ss[:], mybir.ActivationFunctionType.Ln)
        YB = 7
        for g0 in range(0, G, YB):
            gb = min(YB, G - g0)
            yt = sb.tile([P, YB, C], f32, tag="yt")
            for i in range(gb):
                nc.vector.tensor_scalar(
                    out=yt[:, i, :], in0=zs[:, g0 + i, :],
                    scalar1=lg[:, g0 + i:g0 + i + 1],
                    scalar2=None, op0=mybir.AluOpType.subtract)
            nc.sync.dma_start(
                y_out[g0 * P:(g0 + gb) * P, :].rearrange(
                    "(t p) w -> p t w", p=P), yt[:, :gb, :])

    nc.compile()
    return nc


def _perm(h, dim):
    # new position d*h_total + hh  <- old position hh*dim + d
    p = np.empty(h * dim, dtype=np.int64)
    for hh in range(h):
        for d in range(dim):
            p[d * h + hh] = hh * dim + d
    return p


def prepare(x, src, dst, W1s, W1d, al1, ar1, b1, W2s, W2d, al2, ar2, b2):
    """Host prep: schedule, constants, program build. Returns (nc, in_maps)."""
    global _LAST_SCHED
    x = np.asarray(x, dtype=np.float32)
    src = np.asarray(src, dtype=np.int32)
    dst = np.asarray(dst, dtype=np.int32)

    sch = _schedule(src, dst)
    _LAST_SCHED = sch
    KA, KB = sch["KA"], sch["KB"]

    perm1 = _perm(H, D)   # 128 -> d-major
    perm2 = _perm(H, C)   # 188 -> c-major

    W1s = np.asarray(W1s, np.float32)
    W1d = np.asarray(W1d, np.float32)
    W2s = np.asarray(W2s, np.float32)
    W2d = np.asarray(W2d, np.float32)
    wel1 = W1s @ _blockdiag(np.asarray(al1, np.float32), HD, H, D)
    wer1 = W1d @ _blockdiag(np.asarray(ar1, np.float32), HD, H, D)
    w1c = np.concatenate([W1s[:, perm1], wel1, wer1], axis=1).astype(BF16)
    # h1 is d-major: permute W2 rows; fs2 output is c-major: permute cols
    wel2 = W2s @ _blockdiag(np.asarray(al2, np.float32), HC, H, C)
    wer2 = W2d @ _blockdiag(np.asarray(ar2, np.float32), HC, H, C)
    w2c = np.concatenate(
        [W2s[:, perm2], wel2, wer2], axis=1)[perm1, :].astype(BF16)

    iota_r = np.tile(np.arange(P, dtype=np.float32), (P, 1)).astype(BF16)
    K = KA + KB
    iota_mc = np.tile(np.repeat(np.arange(P, dtype=np.float32), K)[None, :],
                      (P, 1)).astype(BF16)
    iota_c = np.arange(P, dtype=np.float32)[:, None]
    ident_np = np.eye(P, dtype=np.float32).astype(BF16)
    b1_np = np.tile(np.asarray(b1, np.float32)[perm1][None, :], (P, 1))
    b2m_np = np.tile(np.asarray(b2, np.float32).reshape(H, C).mean(0)[None, :],
                     (P, 1))

    # xT in table-row order, replicated; xT_own per core in local order
    xb = x.astype(BF16)
    xT_all = np.zeros((P, NTAB), BF16)
    rows = _row_of(np.arange(N, dtype=np.int64))
    xT_all[:, rows] = xb.T
    xT_own = np.zeros((NCORES, P, NPAD), BF16)
    for c in range(NCORES):
        xT_own[c, :, :NPC] = xb[c * NPC:(c + 1) * NPC].T

    nc = _build_program(KA, KB)

    in_maps = []
    for c in range(NCORES):
        in_maps.append({
            "xT_all": xT_all,
            "xT_own": xT_own[c],
            "w1cat": w1c, "w2cat": w2c,
            "b1_t": b1_np, "b2m_t": b2m_np,
            "iota_r": iota_r, "iota_c": iota_c, "ident_t": ident_np,
            "iota_mc": iota_mc,
            "sched_in": sch["sched"][c],
            "dstl_row_in": sch["dstl_row"][c],
        })
    return nc, in_maps


def kernel(x, src, dst, W1s, W1d, al1, ar1, b1, W2s, W2d, al2, ar2, b2):
    global LAST_EXEC_NS
    nc, in_maps = prepare(x, src, dst, W1s, W1d, al1, ar1, b1,
                          W2s, W2d, al2, ar2, b2)
    res = run_bass_kernel_spmd(nc, in_maps, list(range(NCORES)))
    LAST_EXEC_NS = res.exec_time_ns
    out = np.concatenate(
        [res.results[c]["y_out"][:NPC] for c in range(NCORES)], axis=0)
    return out.astype(np.float32)
